# revision 44
# baseline (speedup 1.0000x reference)
"""Trainium2 Bass kernel for nn_G3DCrossAttention (B=2, C=512, L=2048, G=2048, H=8).

Math (exact, same collapse as the v0 kernel, restructured for latency):
  exp_p[g,b,:] = exp[b,g]*Wg[:,0] + bg is rank-1 in channels, so attention
  collapses to x_attn = w*u_v + c_v per head with w = f_b(a), where
  a = x_seq @ M + a0 (M, a0, u_v, c_v are weight-only quantities, computed on
  host), and f_b is a smooth per-batch scalar function. f_b is sampled exactly
  at 64 Chebyshev nodes per batch on-device (exp + weighted sums over all
  G=2048 e_j), fit with a degree-14 Chebyshev series (one small PE matmul),
  and evaluated by an even/odd-split dual Clenshaw recurrence (depth 7+7) on
  the vector engine.

Performance structure vs v0 (180us):
  - No DRAM round-trips on the critical path: the a->packed-layout repack, the
    coefficient broadcast, and the w->[c,t] replication all run on the PE
    (transposes / K<=8 matmuls) instead of DMA bounces (~4-5us latency each).
  - LayerNorm gains/biases folded into W1/Wo on host; per-token stats use fp16
    matmuls against early-computed xc = c_v + x_seq terms plus tiny
    w-dependent corrections, then rstd is computed with a PE row-broadcast +
    128-partition vector reciprocal (the v0 single-partition reciprocal cost
    3.3us each).
  - All input DMAs are issued at t=0 across 4 queues.

Sharding: data-parallel over L across 8 cores (L/8 = 256 queries each).
"""

from contextlib import ExitStack

import numpy as np

import concourse.bass as bass
import concourse.tile as tile
from concourse import bacc, mybir
from concourse.bass_utils import run_bass_kernel_spmd

F32 = mybir.dt.float32
F32R = mybir.dt.float32r
FP16 = mybir.dt.float16
AF = mybir.ActivationFunctionType
OP = mybir.AluOpType

B, C, L, G, H = 2, 512, 2048, 2048, 8
D = C // H
NCORES = 8
LC = L // NCORES              # 256 queries per core
T = B * LC                    # 512 tokens per core (tau = b*LC + l)
KC = C // 128                 # 4 partition tiles over C
KH = (4 * C) // 128           # 16 partition tiles over 4C
SCALE = 1.0 / float(np.sqrt(D))
EPS = 1e-5
SCAL = 5.0                    # Chebyshev half-range in a-units (|a|max ~ 4.43)
KDEG = 14                     # Chebyshev series length (w err ~1.6e-3)
NE = KDEG // 2                # even coeffs c0,c2,..,c12
NO = KDEG // 2                # odd coeffs c1,c3,..,c13
MNODES = 32                   # Chebyshev nodes/batch; genes split over 2 halves
GCH = 4                       # node-eval chunks over G
GC = G // GCH

TRACE = False
TRACE_KW = {}
LAST_RESULTS = None

_CACHE = None


def _consts():
    # node layout: p = b*64 + gh*32 + m  (gh = gene half, m = node 0..31)
    m = np.arange(MNODES)
    theta = np.pi * (2 * m + 1) / (2 * MNODES)
    xn32 = (SCAL * np.cos(theta)).astype(np.float32)
    xnodes = np.concatenate([xn32, xn32, xn32, xn32])[:, None]   # [128,1]
    # dct32[q,k] for q = b*32 + m: (2/MN)*cos(k*theta_m), k=0 halved
    dct_c = np.zeros((2 * MNODES, KDEG), np.float32)
    for k in range(KDEG):
        dct_c[:MNODES, k] = (2.0 / MNODES) * np.cos(k * theta)
        dct_c[MNODES:, k] = dct_c[:MNODES, k]
    dct_c[:, 0] *= 0.5
    bi = np.zeros((2 * MNODES, 2), np.float32)              # [q, b] indicator
    bi[:MNODES, 0] = 1.0
    bi[MNODES:, 1] = 1.0
    bgh = np.zeros((2, 2 * 128), np.float32)                # [b, (gh,p)]
    cmb = np.zeros((128, 2 * MNODES), np.float32)           # [p, (b,m)]
    for p in range(128):
        b, gh, mm_ = p // 64, (p // 32) % 2, p % 32
        bgh[b, gh * 128 + p] = 1.0
        cmb[p, b * MNODES + mm_] = 1.0
    bi2 = np.zeros((2, 128), np.float32)                    # [b, p] for cb
    bi2[0, :64] = 1.0
    bi2[1, 64:] = 1.0
    identrep = np.eye(128, dtype=np.float32)                # PE transpose identity
    ident16 = np.eye(128, dtype=np.float16)
    ohot = np.zeros((4, 4 * 128), np.float16)               # row-select lhsT
    for r in range(4):
        ohot[r, r * 128:(r + 1) * 128] = 1.0
    return xnodes, dct_c, bi, bi2, identrep, ident16, ohot, bgh, cmb


def _build():
    nc = bacc.Bacc(debug=False, num_devices=NCORES)

    # ---- external inputs -------------------------------------------------
    seq_sl = nc.dram_tensor("seq_sl", [B, C, LC], F32, kind="ExternalInput")
    expv = nc.dram_tensor("expv", [B, G], F32, kind="ExternalInput")
    w1t = nc.dram_tensor("w1t", [C, 4 * C], FP16, kind="ExternalInput")   # (W1*g1).T
    w2t = nc.dram_tensor("w2t", [4 * C, C], FP16, kind="ExternalInput")   # W2.T
    wot = nc.dram_tensor("wot", [C, C], FP16, kind="ExternalInput")       # (Wo*g2).T
    mp = nc.dram_tensor("mp", [128, KC * H], F32, kind="ExternalInput")   # M' packed
    a0p = nc.dram_tensor("a0p", [H, 1], F32, kind="ExternalInput")        # a0'
    uvrep = nc.dram_tensor("uvrep", [H, C], FP16, kind="ExternalInput")   # u_v rep
    uvmask = nc.dram_tensor("uvmask", [128, KC * H], FP16, kind="ExternalInput")
    spack = nc.dram_tensor("spack", [H, 3], FP16, kind="ExternalInput")   # s_uv/C, s_uv2/C, 2/C
    vecp = nc.dram_tensor("vecp", [128, KC * 5], F32, kind="ExternalInput")  # c_v,b2,bof,g1,be1
    b1fp = nc.dram_tensor("b1fp", [128, KH], F32, kind="ExternalInput")   # b1 + W1g@be1

    out_sl = nc.dram_tensor("out_sl", [B, C, LC], F32, kind="ExternalOutput")

    # ---- inline constants ------------------------------------------------
    (xn_np, dct_np, bi_np, bi2_np, ident_np, id16_np, ohot_np,
     bgh_np, cmb_np) = _consts()
    c_xn = nc.inline_tensor(xn_np, name="c_xn")
    c_dct = nc.inline_tensor(dct_np, name="c_dct")
    c_bi = nc.inline_tensor(bi_np, name="c_bi")
    c_bi2 = nc.inline_tensor(bi2_np, name="c_bi2")
    c_ident = nc.inline_tensor(ident_np, name="c_ident")
    c_id16 = nc.inline_tensor(id16_np, name="c_id16")
    c_ohot = nc.inline_tensor(ohot_np, name="c_ohot")
    c_bgh = nc.inline_tensor(bgh_np, name="c_bgh")
    c_cmb = nc.inline_tensor(cmb_np, name="c_cmb")

    with tile.TileContext(nc) as tc, ExitStack() as ctx:
        p_w1 = ctx.enter_context(tc.tile_pool(name="w1", bufs=KC))
        p_w2 = ctx.enter_context(tc.tile_pool(name="w2", bufs=1))
        p_wo = ctx.enter_context(tc.tile_pool(name="wo", bufs=1))
        p_xs = ctx.enter_context(tc.tile_pool(name="xs", bufs=4))
        p_h = ctx.enter_context(tc.tile_pool(name="h", bufs=16))
        p_act = ctx.enter_context(tc.tile_pool(name="act", bufs=4))
        p_nd = ctx.enter_context(tc.tile_pool(name="nd", bufs=1))
        p_sm = ctx.enter_context(tc.tile_pool(name="sm", bufs=1))
        p_cl = ctx.enter_context(tc.tile_pool(name="cl", bufs=1))
        ps = ctx.enter_context(tc.tile_pool(name="ps", bufs=1, space="PSUM"))

        # =========== t=0: DMA issues, spread over queues ==================
        # sync queue: xs tiles first, then M' pack, identities, W1
        xs_all = p_xs.tile([128, KC * T], F32R, tag="xs")
        for b in range(B):
            nc.sync.dma_start(
                xs_all[:].rearrange("p (kt b l) -> p kt b l", kt=KC, b=B)[:, :, b, :],
                seq_sl[b, :, :].rearrange("(kt p) l -> p kt l", p=128).bitcast(F32R))
        xs_t = [xs_all[:, kt * T:(kt + 1) * T] for kt in range(KC)]
        mp_sb = p_sm.tile([128, KC * H], F32R, tag="mp")
        nc.sync.dma_start(mp_sb[:], mp[:].bitcast(F32R))
        ident_sb = p_sm.tile([128, 128], F32, tag="ident")
        nc.sync.dma_start(ident_sb[:], c_ident[:])
        id16_sb = p_sm.tile([128, 128], FP16, tag="id16")
        nc.sync.dma_start(id16_sb[:], c_id16[:])
        w1_t = [p_w1.tile([128, 4 * C], FP16, tag="w1", name=f"w1_{i}")
                for i in range(KC)]
        for kt in range(KC):
            nc.sync.dma_start(w1_t[kt][:], w1t[kt * 128:(kt + 1) * 128, :])

        # scalar queue: exp + small consts (e-broadcast happens on the PE)
        exp_sb = p_sm.tile([2, G], F32R, tag="expsb")
        nc.scalar.dma_start(exp_sb[:], expv[:].bitcast(F32R))
        xn_col = p_sm.tile([128, 1], F32, tag="xn")
        nc.scalar.dma_start(xn_col[:], c_xn[:])
        dct_sb = p_sm.tile([2 * MNODES, KDEG], F32, tag="dct")
        nc.scalar.dma_start(dct_sb[:], c_dct[:])

        # gpsimd queue: memsets, consts, W2, Wo
        ones_pe = p_sm.tile([1, 128], FP16, tag="onespe")
        nc.gpsimd.memset(ones_pe[:], 1.0)
        nones_pe = p_sm.tile([1, 128], FP16, tag="nonespe")
        nc.gpsimd.memset(nones_pe[:], -1.0)
        ones_c = p_sm.tile([128, 1], FP16, tag="onesc")
        nc.gpsimd.memset(ones_c[:], 1.0 / C)
        ohot_sb = p_sm.tile([4, 4 * 128], FP16, tag="ohot")
        nc.gpsimd.dma_start(ohot_sb[:], c_ohot[:])

        eps_col = p_sm.tile([128, 1], F32, tag="epsc")
        nc.gpsimd.memset(eps_col[:], EPS)
        bi2_sb = p_sm.tile([2, 128], F32R, tag="bi2")
        nc.gpsimd.dma_start(bi2_sb[:], c_bi2[:].bitcast(F32R))
        vec_sb = p_sm.tile([128, KC * 5], F32, tag="vecp")
        nc.gpsimd.dma_start(vec_sb[:], vecp[:])
        a0_sb = p_sm.tile([H, 1], F32, tag="a0")
        nc.gpsimd.dma_start(a0_sb[:], a0p[:])
        bi_sb = p_sm.tile([2 * MNODES, 2], F32R, tag="bi")
        nc.gpsimd.dma_start(bi_sb[:], c_bi[:].bitcast(F32R))
        bgh_sb = p_sm.tile([2, 2 * 128], F32R, tag="bgh")
        nc.gpsimd.dma_start(bgh_sb[:], c_bgh[:].bitcast(F32R))
        cmb_sb = p_sm.tile([128, 2 * MNODES], F32, tag="cmb")
        nc.gpsimd.dma_start(cmb_sb[:], c_cmb[:])
        uvrep_sb = p_sm.tile([H, C], FP16, tag="uvrep")
        nc.gpsimd.dma_start(uvrep_sb[:], uvrep[:])
        uvmask_sb = p_sm.tile([128, KC * H], FP16, tag="uvmask")
        nc.gpsimd.dma_start(uvmask_sb[:], uvmask[:])
        spack_sb = p_sm.tile([H, 3], FP16, tag="spack")
        nc.gpsimd.dma_start(spack_sb[:], spack[:])
        b1f_sb = p_sm.tile([128, KH], F32, tag="b1f")
        nc.gpsimd.dma_start(b1f_sb[:], b1fp[:])

        def vcol(i):
            # column i of the packed per-channel vectors, per c-tile kt
            return lambda kt: vec_sb[:, kt * 5 + i:kt * 5 + i + 1]
        cv_c, b2_c, bof_c, g1_c, be1_c = (vcol(i) for i in range(5))

        # =========== node eval: 32 nodes/batch, genes split over rows =====
        # row p = b*64 + gh*32 + m handles genes [gh*1024, gh*1024+1024)
        NE2 = G // 2
        pn = p_nd.tile([128, NE2], F32, tag="pn")
        zc = p_sm.tile([128, 2], F32, tag="zc")
        nmc = p_sm.tile([128, 2], F32, tag="nmc")
        e_ps = []
        for ch in range(2):
            ep = ps.tile([128, 512], F32, tag="mm", bufs=3, name=f"eb{ch}")
            for gh in range(2):
                nc.tensor.matmul(
                    ep[:], bgh_sb[:, gh * 128:(gh + 1) * 128],
                    exp_sb[:, gh * NE2 + ch * 512:gh * NE2 + (ch + 1) * 512],
                    start=(gh == 0), stop=(gh == 1))
            e_ps.append(ep)
        for ch in range(2):
            sl = slice(ch * 512, (ch + 1) * 512)
            nc.scalar.activation(pn[:, sl], e_ps[ch][:], AF.Exp,
                                 scale=xn_col[:],
                                 accum_out=zc[:, ch:ch + 1])
        for ch in range(2):
            sl = slice(ch * 512, (ch + 1) * 512)
            nc.vector.scalar_tensor_tensor(
                out=pn[:, sl], in0=pn[:, sl], scalar=1.0, in1=e_ps[ch][:],
                op0=OP.mult, op1=OP.mult, accum_out=nmc[:, ch:ch + 1])
        # zn[:,0] = z-half, zn[:,1] = nm-half; combine halves on the PE
        zn = p_sm.tile([128, 2], F32, tag="zn")
        nc.vector.tensor_reduce(zn[:, 0:1], zc[:], mybir.AxisListType.X, OP.add)
        nc.vector.tensor_reduce(zn[:, 1:2], nmc[:], mybir.AxisListType.X, OP.add)
        zn2 = ps.tile([2 * MNODES, 2], F32, tag="sb", name="zn2")
        nc.tensor.matmul(zn2[:], cmb_sb[:], zn[:], start=True, stop=True)
        zr_col = p_sm.tile([2 * MNODES, 1], F32, tag="zrc")
        nc.vector.reciprocal(zr_col[:], zn2[:, 0:1])
        f_col = p_sm.tile([2 * MNODES, 1], F32, tag="fc")
        nc.vector.tensor_mul(f_col[:], zn2[:, 1:2], zr_col[:])
        dctf = p_sm.tile([2 * MNODES, KDEG], F32R, tag="dctf")
        nc.vector.tensor_scalar_mul(dctf[:], dct_sb[:], f_col[:])

        # =========== a path -> tt packed [128, 32] via PE transposes ======
        pa = ps.tile([H, T], F32, tag="sa")
        for kt in range(KC):
            nc.tensor.matmul(pa[:], mp_sb[:, kt * H:(kt + 1) * H], xs_t[kt][:],
                             start=(kt == 0), stop=(kt == KC - 1))
        # tt_sb columns permuted to (lh, b, ll): transposing a contiguous
        # [8, 128] block then yields partitions p = b*64 + ll (batch-major),
        # with matmul/psum outputs at partition base 0 as required.
        tt_sb = p_sm.tile([H, T], F32, tag="tts")
        nc.scalar.activation(
            tt_sb[:],
            pa[:].rearrange("h (b lh ll) -> h lh b ll", b=2, lh=4),
            AF.Identity, bias=a0_sb[:])
        tt_ps = ps.tile([128, 32], F32, tag="sb")
        for lh in range(4):
            nc.tensor.transpose(
                tt_ps[:, lh * 8:(lh + 1) * 8],
                tt_sb[0:8, lh * 128:(lh + 1) * 128],
                ident_sb[0:8, 0:8])

        # =========== Chebyshev fit on PE: ck [2,14] -> cbB [128,14] =======
        ck_ps = ps.tile([2, KDEG], F32, tag="sb", name="ckps")
        nc.tensor.matmul(ck_ps[:], bi_sb[:], dctf[:],
                         start=True, stop=True)
        ck_sb = p_sm.tile([2, KDEG], F32R, tag="cksb")
        nc.scalar.copy(ck_sb[:], ck_ps[:])
        cb_ps = ps.tile([128, KDEG], F32, tag="sb", name="cbps")
        nc.tensor.matmul(cb_ps[:], bi2_sb[:], ck_sb[:],
                         start=True, stop=True)
        cb = p_cl.tile([128, KDEG], F32, tag="cb")
        nc.scalar.copy(cb[:], cb_ps[:])

        def ce(k):  # even coeff c_{2k}
            return cb[:, 2 * k:2 * k + 1]

        def co(k):  # odd coeff c_{2k+1}
            return cb[:, 2 * k + 1:2 * k + 2]

        # =========== xc tiles + early stat matmuls ========================
        xc_t, xc2_t = [], []
        for kt in range(KC):
            xc = p_act.tile([128, T], FP16, tag="xc", bufs=4, name=f"xc{kt}")
            nc.scalar.activation(xc[:], xs_t[kt][:].bitcast(F32), AF.Identity,
                                 bias=cv_c(kt))
            xc_t.append(xc)
        # bulk fp16 weight loads, issued behind xc on the gpsimd queue
        w2_all = p_w2.tile([128, KH * C], FP16, tag="w2")
        for kt in range(KH):
            nc.gpsimd.dma_start(w2_all[:, kt * C:(kt + 1) * C],
                                w2t[kt * 128:(kt + 1) * 128, :])
        wo_all = p_wo.tile([128, KC * C], FP16, tag="wo")
        for kt in range(KC):
            nc.gpsimd.dma_start(wo_all[:, kt * C:(kt + 1) * C],
                                wot[kt * 128:(kt + 1) * 128, :])
        def w2_sl(kt, mt):
            return w2_all[:, kt * C + mt * 128:kt * C + (mt + 1) * 128]

        def wo_sl(kt, mt):
            return wo_all[:, kt * C + mt * 128:kt * C + (mt + 1) * 128]
        warm = p_sm.tile([128, 1], F32, tag="warm")
        nc.scalar.activation(warm[:], eps_col[:], AF.Sqrt)
        # xcu[h, tau] = sum_{c in h} u_v[c] xc[c, tau]
        xcu_ps = ps.tile([H, T], F32, tag="sb")
        for kt in range(KC):
            nc.tensor.matmul(xcu_ps[:], uvmask_sb[:, kt * H:(kt + 1) * H],
                             xc_t[kt][:], start=(kt == 0), stop=(kt == KC - 1))
        # stat0/stat1 partial sums over xc / xc^2 (keep groups open for w terms)
        st0 = ps.tile([1, T], F32, tag="st0")
        st1 = ps.tile([1, T], F32, tag="st1")
        for kt in range(KC):
            nc.tensor.matmul(st0[:], ones_c[:], xc_t[kt][:],
                             start=(kt == 0), stop=False)

        # =========== Clenshaw: dual chains on [128, 32] ===================
        tt = p_cl.tile([128, 32], F32, tag="tt")
        nc.vector.tensor_scalar(tt[:], tt_ps[:], -1.0, 1.0, op0=OP.max, op1=OP.min)
        t2 = p_cl.tile([128, 32], F32, tag="t2")
        nc.vector.tensor_mul(t2[:], tt[:], tt[:])
        u2 = p_cl.tile([128, 32], F32, tag="u2")
        nc.vector.tensor_scalar(u2[:], t2[:], 4.0, -2.0, op0=OP.mult, op1=OP.add)
        uu = p_cl.tile([128, 32], F32, tag="uu")
        nc.vector.tensor_scalar(uu[:], t2[:], 2.0, -1.0, op0=OP.mult, op1=OP.add)

        eb1 = p_cl.tile([128, 32], F32, tag="eb1")
        eb2 = p_cl.tile([128, 32], F32, tag="eb2")
        etm = p_cl.tile([128, 32], F32, tag="etm")
        ob1 = p_cl.tile([128, 32], F32, tag="ob1")
        ob2 = p_cl.tile([128, 32], F32, tag="ob2")
        otm = p_cl.tile([128, 32], F32, tag="otm")
        # init: b_{n-2} = c_{n-2} + 2u*c_{n-1}  (b_{n-1}=c_{n-1}, b_n=0)
        nc.vector.tensor_scalar(eb2[:], u2[:], ce(NE - 1), ce(NE - 2),
                                op0=OP.mult, op1=OP.add)
        nc.vector.tensor_scalar(ob2[:], u2[:], co(NO - 1), co(NO - 2),
                                op0=OP.mult, op1=OP.add)
        # eb2 currently holds b_{n-2}; need previous b_{n-1} too
        nc.vector.tensor_scalar(eb1[:], u2[:], 0.0, ce(NE - 1),
                                op0=OP.mult, op1=OP.add)
        nc.vector.tensor_scalar(ob1[:], u2[:], 0.0, co(NO - 1),
                                op0=OP.mult, op1=OP.add)
        # now (cur1, cur2) = (b_{m+1}, b_{m+2}) with m+1 = n-2 held in (eb2, eb1)
        ecur1, ecur2 = eb2, eb1
        ocur1, ocur2 = ob2, ob1
        for k in range(NE - 3, 0, -1):
            nc.vector.tensor_mul(etm[:], u2[:], ecur1[:])
            nc.vector.tensor_mul(otm[:], u2[:], ocur1[:])
            nc.vector.scalar_tensor_tensor(
                out=ecur2[:], in0=etm[:], scalar=ce(k), in1=ecur2[:],
                op0=OP.add, op1=OP.subtract)
            nc.vector.scalar_tensor_tensor(
                out=ocur2[:], in0=otm[:], scalar=co(k), in1=ocur2[:],
                op0=OP.add, op1=OP.subtract)
            ecur1, ecur2 = ecur2, ecur1
            ocur1, ocur2 = ocur2, ocur1
        # even final: fe = u*b1 - b2 + c0
        fe = p_cl.tile([128, 32], F32, tag="fe")
        nc.vector.tensor_mul(etm[:], uu[:], ecur1[:])
        nc.vector.scalar_tensor_tensor(
            out=fe[:], in0=etm[:], scalar=ce(0), in1=ecur2[:],
            op0=OP.add, op1=OP.subtract)
        # odd final: b0 = c0 + 2u*b1 - b2 ; fo = b0 - b1
        nc.vector.tensor_mul(otm[:], u2[:], ocur1[:])
        nc.vector.scalar_tensor_tensor(
            out=ocur2[:], in0=otm[:], scalar=co(0), in1=ocur2[:],
            op0=OP.add, op1=OP.subtract)
        fo = p_cl.tile([128, 32], F32, tag="fo")
        nc.vector.tensor_sub(fo[:], ocur2[:], ocur1[:])
        # w = fe + t*fo
        w_pack = p_cl.tile([128, 32], F32, tag="wp")
        nc.vector.tensor_mul(otm[:], tt[:], fo[:])
        nc.vector.tensor_add(w_pack[:], fe[:], otm[:])

        for kt in range(KC):
            x2 = p_act.tile([128, T], FP16, tag="xc2", bufs=4, name=f"xc2{kt}")
            nc.vector.tensor_mul(x2[:], xc_t[kt][:], xc_t[kt][:])
            xc2_t.append(x2)
        for kt in range(KC):
            nc.tensor.matmul(st1[:], ones_c[:], xc2_t[kt][:],
                             start=(kt == 0), stop=False)

        # =========== w -> [H, T] via reverse transposes ===================
        # wT columns come out in (lh, b, ll) order; the psum->sbuf copies
        # permute back to (b, lh, ll) = tau order.
        wht_ps = ps.tile([H, T], F32, tag="sa", name="whtps")
        for lh in range(4):
            nc.tensor.transpose(
                wht_ps[0:8, lh * 128:(lh + 1) * 128],
                w_pack[:, lh * 8:(lh + 1) * 8],
                ident_sb[:, :])
        wsrc = wht_ps[:].rearrange("h (lh b ll) -> h b lh ll", b=2, lh=4)
        whth = p_sm.tile([H, T], FP16, tag="whth")
        nc.vector.tensor_copy(whth[:], wsrc)
        w2h = p_sm.tile([H, T], FP16, tag="w2h")
        nc.vector.tensor_mul(w2h[:], whth[:], whth[:])
        wxcu = p_sm.tile([H, T], FP16, tag="wxcu")
        nc.vector.tensor_mul(wxcu[:], whth[:], xcu_ps[:])

        # =========== finish LN1 stats with w-dependent terms ==============
        nc.tensor.matmul(st0[:], spack_sb[:, 0:1], whth[:], start=False, stop=True)
        nc.tensor.matmul(st1[:], spack_sb[:, 1:2], w2h[:], start=False, stop=False)
        nc.tensor.matmul(st1[:], spack_sb[:, 2:3], wxcu[:], start=False, stop=True)

        def ln_front(st0_ps, st1_ps, ph):
            """stats psums [1,T] (mu, E[y^2]) -> (smu [1,T] f32r, rstdp
            [128,4] fp16). rstd is computed in a transposed [128,4] packing so
            the reciprocal runs at 4 elems/lane instead of 512."""
            smu = p_sm.tile([1, T], F32, tag=f"smu{ph}")
            nc.scalar.copy(smu[:], st0_ps[:])
            mu16 = p_sm.tile([1, T], FP16, tag=f"mu16{ph}")
            nc.scalar.copy(mu16[:], st0_ps[:])
            ss1 = p_sm.tile([1, T], F32, tag=f"ss1{ph}")
            nc.vector.tensor_copy(ss1[:], st1_ps[:])
            lnpmu = ps.tile([128, 4], F32, tag="sa", name=f"lnpmu{ph}")
            for r in range(4):
                nc.tensor.transpose(lnpmu[:, r:r + 1],
                                    smu[0:1, r * 128:(r + 1) * 128],
                                    ident_sb[0:1, 0:1])
            lnps1 = ps.tile([128, 4], F32, tag="sb", name=f"lnps1{ph}")
            for r in range(4):
                nc.tensor.transpose(lnps1[:, r:r + 1],
                                    ss1[0:1, r * 128:(r + 1) * 128],
                                    ident_sb[0:1, 0:1])
            mup = p_sm.tile([128, 4], F32, tag=f"mup{ph}")
            nc.scalar.copy(mup[:], lnpmu[:])
            musq = p_sm.tile([128, 4], F32, tag=f"msq{ph}")
            nc.vector.tensor_mul(musq[:], mup[:], mup[:])
            var = p_sm.tile([128, 4], F32, tag=f"var{ph}")
            nc.vector.tensor_sub(var[:], lnps1[:], musq[:])
            stdp = p_sm.tile([128, 4], FP16, tag=f"stdp{ph}")
            nc.scalar.activation(stdp[:], var[:], AF.Sqrt, bias=eps_col[:])
            rstdp = p_sm.tile([128, 4], FP16, tag=f"rsp{ph}")
            with nc.allow_low_precision(reason="fp16 rstd is plenty for LN"):
                nc.vector.reciprocal(rstdp[:], stdp[:])
            return mu16, rstdp

        def ln_back(rstdp, ph):
            rsT = ps.tile([4, 128], FP16, tag="sb", name=f"rsT{ph}")
            nc.tensor.transpose(rsT[:], rstdp[:], id16_sb[:, :])
            rsrow = p_sm.tile([4, 128], FP16, tag=f"rsr{ph}")
            nc.scalar.copy(rsrow[:], rsT[:])
            # chunk r of R selects rstd row r via a one-hot lhsT
            rps = ps.tile([128, T], F32, tag="bcR", name=f"rps{ph}")
            for r in range(4):
                nc.tensor.matmul(rps[:, r * 128:(r + 1) * 128],
                                 ohot_sb[:, r * 128:(r + 1) * 128],
                                 rsrow[:], start=True, stop=True)
            return rps

        with tc.high_priority():
            mu16a, rstdp1 = ln_front(st0, st1, "a")
            rps1 = ln_back(rstdp1, "a")

        # =========== ym = y - mu accumulated on the PE; xhat = ym * rstd ==
        xh_t, x_t = [], []
        for kt in range(KC):
            ym = ps.tile([128, T], F32, tag="mm", bufs=3, name=f"ym{kt}")
            nc.tensor.matmul(ym[:], uvrep_sb[:, kt * 128:(kt + 1) * 128],
                             whth[:], start=True, stop=False)
            nc.tensor.matmul(ym[:], id16_sb[:, :], xc_t[kt][:],
                             start=False, stop=False)
            nc.tensor.matmul(ym[:], nones_pe[:], mu16a[:],
                             start=False, stop=True)
            ysb = p_act.tile([128, T], FP16, tag="ysb", bufs=4, name=f"ysb{kt}")
            nc.scalar.copy(ysb[:], ym[:])
            xh = p_act.tile([128, T], FP16, tag="xh", bufs=4, name=f"xh{kt}")
            nc.vector.tensor_mul(xh[:], ysb[:], rps1[:])
            xh_t.append(xh)
        # x = g1*xhat + be1 (residual input), off critical path
        for kt in range(KC):
            xk = p_act.tile([128, T], FP16, tag="x", bufs=4, name=f"x{kt}")
            nc.scalar.activation(xk[:], xh_t[kt][:], AF.Identity,
                                 scale=g1_c(kt), bias=be1_c(kt))
            x_t.append(xk)

        # =========== FFN1: h = relu(W1g @ xhat + b1f) =====================
        h_t = []
        for mt in range(KH):
            sl = slice(mt * 128, (mt + 1) * 128)
            pf = ps.tile([128, T], F32, tag="mm", bufs=3, name=f"pf1{mt}")
            for kt in range(KC):
                nc.tensor.matmul(pf[:], w1_t[kt][:, sl], xh_t[kt][:],
                                 start=(kt == 0), stop=(kt == KC - 1))
            hm = p_h.tile([128, T], FP16, tag="h", name=f"h{mt}")
            nc.scalar.activation(hm[:], pf[:], AF.Relu,
                                 bias=b1f_sb[:, mt:mt + 1])
            h_t.append(hm)

        # =========== FFN2 + residual -> y2 ================================
        y2_t, sq2_t = [], []
        for mt in range(KC):
            sl = slice(mt * 128, (mt + 1) * 128)
            pf = ps.tile([128, T], F32, tag="mm", bufs=3, name=f"pf2{mt}")
            for kt in range(KH):
                nc.tensor.matmul(pf[:], w2_sl(kt, mt), h_t[kt][:],
                                 start=(kt == 0), stop=(kt == KH - 1))
            y2 = p_act.tile([128, T], FP16, tag="y2", bufs=4, name=f"y2{mt}")
            nc.vector.scalar_tensor_tensor(
                out=y2[:], in0=x_t[mt][:], scalar=b2_c(mt), in1=pf[:],
                op0=OP.add, op1=OP.add)
            y2_t.append(y2)
            sq = p_act.tile([128, T], FP16, tag="sq2", bufs=4, name=f"sq2{mt}")
            nc.vector.tensor_mul(sq[:], y2[:], y2[:])
            sq2_t.append(sq)

        # =========== LN2 stats + zhat =====================================
        st0b = ps.tile([1, T], F32, tag="st0", name="st0b")
        st1b = ps.tile([1, T], F32, tag="st1", name="st1b")
        for kt in range(KC):
            nc.tensor.matmul(st0b[:], ones_c[:], y2_t[kt][:],
                             start=(kt == 0), stop=(kt == KC - 1))
        for kt in range(KC):
            nc.tensor.matmul(st1b[:], ones_c[:], sq2_t[kt][:],
                             start=(kt == 0), stop=(kt == KC - 1))
        with tc.high_priority():
            mu16b, rstdp2 = ln_front(st0b, st1b, "b")
            rps2 = ln_back(rstdp2, "b")
        zh_t = []
        for kt in range(KC):
            ym2 = ps.tile([128, T], F32, tag="mm", bufs=3, name=f"ym2{kt}")
            nc.tensor.matmul(ym2[:], id16_sb[:, :], y2_t[kt][:],
                             start=True, stop=False)
            nc.tensor.matmul(ym2[:], nones_pe[:], mu16b[:],
                             start=False, stop=True)
            y2sb = p_act.tile([128, T], FP16, tag="ysb", bufs=4, name=f"y2sb{kt}")
            nc.scalar.copy(y2sb[:], ym2[:])
            zh = p_act.tile([128, T], FP16, tag="zh", bufs=4, name=f"zh{kt}")
            nc.vector.tensor_mul(zh[:], y2sb[:], rps2[:])
            zh_t.append(zh)

        # =========== out = Wo_g @ zhat + bof ==============================
        oq = [nc.sync, nc.scalar, nc.sync, nc.scalar]
        for mt in range(KC):
            sl = slice(mt * 128, (mt + 1) * 128)
            pf = ps.tile([128, T], F32, tag="mm", bufs=3, name=f"pfo{mt}")
            for kt in range(KC):
                nc.tensor.matmul(pf[:], wo_sl(kt, mt), zh_t[kt][:],
                                 start=(kt == 0), stop=(kt == KC - 1))
            om = p_act.tile([128, T], F32, tag="om", bufs=2, name=f"om{mt}")
            for b in range(B):
                sl2 = slice(b * LC, (b + 1) * LC)
                if (2 * mt + b) % 2 == 0:
                    nc.scalar.activation(om[:, sl2], pf[:, sl2], AF.Identity,
                                         bias=bof_c(mt))
                else:
                    nc.vector.tensor_scalar_add(om[:, sl2], pf[:, sl2],
                                                bof_c(mt))
                dq = nc.sync if b == 0 else nc.scalar
                dq.dma_start(out_sl[b, mt * 128:(mt + 1) * 128, :], om[:, sl2])

    nc.compile()
    return nc


def _host_prep(inputs):
    f32 = lambda x: np.ascontiguousarray(np.asarray(x), dtype=np.float32)
    f16 = lambda x: np.ascontiguousarray(np.asarray(x), dtype=np.float16)
    Wq, Wk, Wv = f32(inputs["Wq"]), f32(inputs["Wk"]), f32(inputs["Wv"])
    Wg, bg = f32(inputs["Wg"]), f32(inputs["bg"])
    bq, bv = f32(inputs["bq"]), f32(inputs["bv"])
    W1, b1, W2, b2 = (f32(inputs["W1"]), f32(inputs["b1"]),
                      f32(inputs["W2"]), f32(inputs["b2"]))
    Wo, bo = f32(inputs["Wo"]), f32(inputs["bo"])
    g1, be1 = f32(inputs["g1"]), f32(inputs["beta1"])
    g2, be2 = f32(inputs["g2"]), f32(inputs["beta2"])

    u_k = Wk @ Wg[:, 0]
    u_v = Wv @ Wg[:, 0]
    c_v = Wv @ bg + bv
    hidx = np.arange(C) // D                       # head of channel c
    U = np.zeros((C, H), np.float32)
    U[np.arange(C), hidx] = u_k
    Mp = (Wq.T @ U) * (SCALE / SCAL)               # [C, H]
    a0 = (bq @ U) * (SCALE / SCAL)                 # [H]

    # packed M': [128, (kt, h)]
    mp = np.zeros((128, KC * H), np.float32)
    for kt in range(KC):
        mp[:, kt * H:(kt + 1) * H] = Mp[kt * 128:(kt + 1) * 128, :]
    uvrep = np.zeros((H, C), np.float32)
    uvrep[hidx, np.arange(C)] = u_v
    uvmask = np.zeros((128, KC * H), np.float32)
    UV = np.zeros((C, H), np.float32)
    UV[np.arange(C), hidx] = u_v
    for kt in range(KC):
        uvmask[:, kt * H:(kt + 1) * H] = UV[kt * 128:(kt + 1) * 128, :]
    s_uv = UV.sum(axis=0) / C                      # [H]
    s_uv2 = (UV * UV).sum(axis=0) / C
    spack = np.stack([s_uv, s_uv2, np.full(H, 2.0 / C)], axis=1)

    W1f = W1 * g1[None, :]
    b1f = b1 + W1 @ be1
    Wof = Wo * g2[None, :]
    bof = bo + Wo @ be2

    vecp = np.zeros((128, KC * 5), np.float32)
    for kt in range(KC):
        sl = slice(kt * 128, (kt + 1) * 128)
        for i, v in enumerate([c_v, b2, bof, g1, be1]):
            vecp[:, kt * 5 + i] = v[sl]
    b1fp = np.zeros((128, KH), np.float32)
    for mt in range(KH):
        b1fp[:, mt] = b1f[mt * 128:(mt + 1) * 128]

    return {
        "expv": f32(inputs["exp"]),
        "w1t": f16(W1f.T),
        "w2t": f16(W2.T),
        "wot": f16(Wof.T),
        "mp": mp,
        "a0p": np.ascontiguousarray(a0[:, None]),
        "uvrep": f16(uvrep),
        "uvmask": f16(uvmask),
        "spack": f16(spack),
        "vecp": vecp,
        "b1fp": b1fp,
    }


def kernel(**inputs):
    global _CACHE, LAST_RESULTS
    if _CACHE is None:
        _CACHE = _build()
    nc = _CACHE

    base = _host_prep(inputs)
    seq = np.ascontiguousarray(np.asarray(inputs["seq"]), dtype=np.float32)
    in_maps = []
    for c in range(NCORES):
        m = dict(base)
        m["seq_sl"] = np.ascontiguousarray(seq[:, :, c * LC:(c + 1) * LC])
        in_maps.append(m)

    res = run_bass_kernel_spmd(nc, in_maps, list(range(NCORES)), trace=TRACE,
                               **TRACE_KW)
    LAST_RESULTS = res
    out = np.empty((B, C, L), np.float32)
    for c in range(NCORES):
        out[:, :, c * LC:(c + 1) * LC] = res.results[c]["out_sl"]
    return out


# revision 45
# speedup vs baseline: 1.0224x; 1.0224x over previous
"""Trainium2 Bass kernel for nn_G3DCrossAttention (B=2, C=512, L=2048, G=2048, H=8).

Math (exact, same collapse as the v0 kernel, restructured for latency):
  exp_p[g,b,:] = exp[b,g]*Wg[:,0] + bg is rank-1 in channels, so attention
  collapses to x_attn = w*u_v + c_v per head with w = f_b(a), where
  a = x_seq @ M + a0 (M, a0, u_v, c_v are weight-only quantities, computed on
  host), and f_b is a smooth per-batch scalar function. f_b is sampled exactly
  at 64 Chebyshev nodes per batch on-device (exp + weighted sums over all
  G=2048 e_j), fit with a degree-14 Chebyshev series (one small PE matmul),
  and evaluated by an even/odd-split dual Clenshaw recurrence (depth 7+7) on
  the vector engine.

Performance structure vs v0 (180us):
  - No DRAM round-trips on the critical path: the a->packed-layout repack, the
    coefficient broadcast, and the w->[c,t] replication all run on the PE
    (transposes / K<=8 matmuls) instead of DMA bounces (~4-5us latency each).
  - LayerNorm gains/biases folded into W1/Wo on host; per-token stats use fp16
    matmuls against early-computed xc = c_v + x_seq terms plus tiny
    w-dependent corrections, then rstd is computed with a PE row-broadcast +
    128-partition vector reciprocal (the v0 single-partition reciprocal cost
    3.3us each).
  - All input DMAs are issued at t=0 across 4 queues.

Sharding: data-parallel over L across 8 cores (L/8 = 256 queries each).
"""

from contextlib import ExitStack

import numpy as np

import concourse.bass as bass
import concourse.tile as tile
from concourse import bacc, mybir
from concourse.bass_utils import run_bass_kernel_spmd

F32 = mybir.dt.float32
F32R = mybir.dt.float32r
FP16 = mybir.dt.float16
AF = mybir.ActivationFunctionType
OP = mybir.AluOpType

B, C, L, G, H = 2, 512, 2048, 2048, 8
D = C // H
NCORES = 8
LC = L // NCORES              # 256 queries per core
T = B * LC                    # 512 tokens per core (tau = b*LC + l)
KC = C // 128                 # 4 partition tiles over C
KH = (4 * C) // 128           # 16 partition tiles over 4C
SCALE = 1.0 / float(np.sqrt(D))
EPS = 1e-5
SCAL = 5.0                    # Chebyshev half-range in a-units (|a|max ~ 4.43)
KDEG = 14                     # Chebyshev series length (w err ~1.6e-3)
NE = KDEG // 2                # even coeffs c0,c2,..,c12
NO = KDEG // 2                # odd coeffs c1,c3,..,c13
MNODES = 32                   # Chebyshev nodes/batch; genes split over 2 halves
GCH = 4                       # node-eval chunks over G
GC = G // GCH

TRACE = False
TRACE_KW = {}
LAST_RESULTS = None

_CACHE = None


def _consts():
    # node layout: p = b*64 + gh*32 + m  (gh = gene half, m = node 0..31)
    m = np.arange(MNODES)
    theta = np.pi * (2 * m + 1) / (2 * MNODES)
    xn32 = (SCAL * np.cos(theta)).astype(np.float32)
    xnodes = np.concatenate([xn32, xn32, xn32, xn32])[:, None]   # [128,1]
    # dct32[q,k] for q = b*32 + m: (2/MN)*cos(k*theta_m), k=0 halved
    dct_c = np.zeros((2 * MNODES, KDEG), np.float32)
    for k in range(KDEG):
        dct_c[:MNODES, k] = (2.0 / MNODES) * np.cos(k * theta)
        dct_c[MNODES:, k] = dct_c[:MNODES, k]
    dct_c[:, 0] *= 0.5
    bi = np.zeros((2 * MNODES, 2), np.float32)              # [q, b] indicator
    bi[:MNODES, 0] = 1.0
    bi[MNODES:, 1] = 1.0
    bgh = np.zeros((2, 2 * 128), np.float32)                # [b, (gh,p)]
    cmb = np.zeros((128, 2 * MNODES), np.float32)           # [p, (b,m)]
    for p in range(128):
        b, gh, mm_ = p // 64, (p // 32) % 2, p % 32
        bgh[b, gh * 128 + p] = 1.0
        cmb[p, b * MNODES + mm_] = 1.0
    bi2 = np.zeros((2, 128), np.float32)                    # [b, p] for cb
    bi2[0, :64] = 1.0
    bi2[1, 64:] = 1.0
    identrep = np.eye(128, dtype=np.float32)                # PE transpose identity
    ident16 = np.eye(128, dtype=np.float16)
    ohot = np.zeros((4, 4 * 128), np.float16)               # row-select lhsT
    for r in range(4):
        ohot[r, r * 128:(r + 1) * 128] = 1.0
    return xnodes, dct_c, bi, bi2, identrep, ident16, ohot, bgh, cmb


def _build():
    nc = bacc.Bacc(debug=False, num_devices=NCORES)

    # ---- external inputs -------------------------------------------------
    seq_sl = nc.dram_tensor("seq_sl", [B, C, LC], F32, kind="ExternalInput")
    expv = nc.dram_tensor("expv", [B, G], F32, kind="ExternalInput")
    w1t = nc.dram_tensor("w1t", [C, 4 * C], FP16, kind="ExternalInput")   # (W1*g1).T
    w2t = nc.dram_tensor("w2t", [4 * C, C], FP16, kind="ExternalInput")   # W2.T
    wot = nc.dram_tensor("wot", [C, C], FP16, kind="ExternalInput")       # (Wo*g2).T
    mp = nc.dram_tensor("mp", [128, KC * H], F32, kind="ExternalInput")   # M' packed
    a0p = nc.dram_tensor("a0p", [H, 1], F32, kind="ExternalInput")        # a0'
    uvrep = nc.dram_tensor("uvrep", [H, C], FP16, kind="ExternalInput")   # u_v rep
    uvmask = nc.dram_tensor("uvmask", [128, KC * H], FP16, kind="ExternalInput")
    spack = nc.dram_tensor("spack", [H, 3], FP16, kind="ExternalInput")   # s_uv/C, s_uv2/C, 2/C
    vecp = nc.dram_tensor("vecp", [128, KC * 5], F32, kind="ExternalInput")  # c_v,b2,bof,g1,be1
    b1fp = nc.dram_tensor("b1fp", [128, KH], F32, kind="ExternalInput")   # b1 + W1g@be1

    out_sl = nc.dram_tensor("out_sl", [B, C, LC], F32, kind="ExternalOutput")

    # ---- inline constants ------------------------------------------------
    (xn_np, dct_np, bi_np, bi2_np, ident_np, id16_np, ohot_np,
     bgh_np, cmb_np) = _consts()
    c_xn = nc.inline_tensor(xn_np, name="c_xn")
    c_dct = nc.inline_tensor(dct_np, name="c_dct")
    c_bi = nc.inline_tensor(bi_np, name="c_bi")
    c_bi2 = nc.inline_tensor(bi2_np, name="c_bi2")
    c_ident = nc.inline_tensor(ident_np, name="c_ident")
    c_id16 = nc.inline_tensor(id16_np, name="c_id16")
    c_ohot = nc.inline_tensor(ohot_np, name="c_ohot")
    c_bgh = nc.inline_tensor(bgh_np, name="c_bgh")
    c_cmb = nc.inline_tensor(cmb_np, name="c_cmb")

    with tile.TileContext(nc) as tc, ExitStack() as ctx:
        p_w1 = ctx.enter_context(tc.tile_pool(name="w1", bufs=KC))
        p_w2 = ctx.enter_context(tc.tile_pool(name="w2", bufs=1))
        p_wo = ctx.enter_context(tc.tile_pool(name="wo", bufs=1))
        p_xs = ctx.enter_context(tc.tile_pool(name="xs", bufs=4))
        p_h = ctx.enter_context(tc.tile_pool(name="h", bufs=16))
        p_act = ctx.enter_context(tc.tile_pool(name="act", bufs=4))
        p_nd = ctx.enter_context(tc.tile_pool(name="nd", bufs=1))
        p_sm = ctx.enter_context(tc.tile_pool(name="sm", bufs=1))
        p_cl = ctx.enter_context(tc.tile_pool(name="cl", bufs=1))
        ps = ctx.enter_context(tc.tile_pool(name="ps", bufs=1, space="PSUM"))

        # =========== t=0: DMA issues, spread over queues ==================
        # sync queue: xs tiles first, then M' pack, identities, W1
        xs_all = p_xs.tile([128, KC * T], F32R, tag="xs")
        for b in range(B):
            nc.sync.dma_start(
                xs_all[:].rearrange("p (kt b l) -> p kt b l", kt=KC, b=B)[:, :, b, :],
                seq_sl[b, :, :].rearrange("(kt p) l -> p kt l", p=128).bitcast(F32R))
        xs_t = [xs_all[:, kt * T:(kt + 1) * T] for kt in range(KC)]
        mp_sb = p_sm.tile([128, KC * H], F32R, tag="mp")
        nc.sync.dma_start(mp_sb[:], mp[:].bitcast(F32R))
        ident_sb = p_sm.tile([128, 128], F32, tag="ident")
        nc.sync.dma_start(ident_sb[:], c_ident[:])
        id16_sb = p_sm.tile([128, 128], FP16, tag="id16")
        nc.sync.dma_start(id16_sb[:], c_id16[:])
        w1_t = [p_w1.tile([128, 4 * C], FP16, tag="w1", name=f"w1_{i}")
                for i in range(KC)]
        for kt in range(KC):
            nc.sync.dma_start(w1_t[kt][:], w1t[kt * 128:(kt + 1) * 128, :])

        # scalar queue: exp + small consts (e-broadcast happens on the PE)
        exp_sb = p_sm.tile([2, G], F32R, tag="expsb")
        nc.scalar.dma_start(exp_sb[:], expv[:].bitcast(F32R))
        xn_col = p_sm.tile([128, 1], F32, tag="xn")
        nc.scalar.dma_start(xn_col[:], c_xn[:])
        dct_sb = p_sm.tile([2 * MNODES, KDEG], F32, tag="dct")
        nc.scalar.dma_start(dct_sb[:], c_dct[:])

        # gpsimd queue: memsets, consts, W2, Wo
        ones_pe = p_sm.tile([1, 128], FP16, tag="onespe")
        nc.gpsimd.memset(ones_pe[:], 1.0)
        nones_pe = p_sm.tile([1, 128], FP16, tag="nonespe")
        nc.gpsimd.memset(nones_pe[:], -1.0)
        ones_c = p_sm.tile([128, 1], FP16, tag="onesc")
        nc.gpsimd.memset(ones_c[:], 1.0 / C)
        ohot_sb = p_sm.tile([4, 4 * 128], FP16, tag="ohot")
        nc.gpsimd.dma_start(ohot_sb[:], c_ohot[:])

        eps_col = p_sm.tile([128, 1], F32, tag="epsc")
        nc.gpsimd.memset(eps_col[:], EPS)
        bgh_sb = p_sm.tile([2, 2 * 128], F32R, tag="bgh")
        nc.gpsimd.dma_start(bgh_sb[:], c_bgh[:].bitcast(F32R))
        vec_sb = p_sm.tile([128, KC * 5], F32, tag="vecp")
        nc.gpsimd.dma_start(vec_sb[:], vecp[:])
        a0_sb = p_sm.tile([H, 1], F32, tag="a0")
        nc.gpsimd.dma_start(a0_sb[:], a0p[:])
        cmb_sb = p_sm.tile([128, 2 * MNODES], F32, tag="cmb")
        nc.gpsimd.dma_start(cmb_sb[:], c_cmb[:])
        bi_sb = p_sm.tile([2 * MNODES, 2], F32R, tag="bi")
        nc.gpsimd.dma_start(bi_sb[:], c_bi[:].bitcast(F32R))
        bi2_sb = p_sm.tile([2, 128], F32R, tag="bi2")
        nc.gpsimd.dma_start(bi2_sb[:], c_bi2[:].bitcast(F32R))
        uvrep_sb = p_sm.tile([H, C], FP16, tag="uvrep")
        nc.gpsimd.dma_start(uvrep_sb[:], uvrep[:])
        uvmask_sb = p_sm.tile([128, KC * H], FP16, tag="uvmask")
        nc.gpsimd.dma_start(uvmask_sb[:], uvmask[:])
        spack_sb = p_sm.tile([H, 3], FP16, tag="spack")
        nc.gpsimd.dma_start(spack_sb[:], spack[:])
        b1f_sb = p_sm.tile([128, KH], F32, tag="b1f")
        nc.gpsimd.dma_start(b1f_sb[:], b1fp[:])

        def vcol(i):
            # column i of the packed per-channel vectors, per c-tile kt
            return lambda kt: vec_sb[:, kt * 5 + i:kt * 5 + i + 1]
        cv_c, b2_c, bof_c, g1_c, be1_c = (vcol(i) for i in range(5))

        # =========== node eval: 32 nodes/batch, genes split over rows =====
        # row p = b*64 + gh*32 + m handles genes [gh*1024, gh*1024+1024)
        NE2 = G // 2
        pn = p_nd.tile([128, NE2], F32, tag="pn")
        zc = p_sm.tile([128, 2], F32, tag="zc")
        nmc = p_sm.tile([128, 2], F32, tag="nmc")
        e_ps = []
        for ch in range(2):
            ep = ps.tile([128, 512], F32, tag="mm", bufs=3, name=f"eb{ch}")
            for gh in range(2):
                nc.tensor.matmul(
                    ep[:], bgh_sb[:, gh * 128:(gh + 1) * 128],
                    exp_sb[:, gh * NE2 + ch * 512:gh * NE2 + (ch + 1) * 512],
                    start=(gh == 0), stop=(gh == 1))
            e_ps.append(ep)
        for ch in range(2):
            sl = slice(ch * 512, (ch + 1) * 512)
            nc.scalar.activation(pn[:, sl], e_ps[ch][:], AF.Exp,
                                 scale=xn_col[:],
                                 accum_out=zc[:, ch:ch + 1])
        for ch in range(2):
            sl = slice(ch * 512, (ch + 1) * 512)
            nc.vector.scalar_tensor_tensor(
                out=pn[:, sl], in0=pn[:, sl], scalar=1.0, in1=e_ps[ch][:],
                op0=OP.mult, op1=OP.mult, accum_out=nmc[:, ch:ch + 1])
        # zn[:,0] = z-half, zn[:,1] = nm-half; combine halves on the PE
        zn = p_sm.tile([128, 2], F32, tag="zn")
        nc.vector.tensor_reduce(zn[:, 0:1], zc[:], mybir.AxisListType.X, OP.add)
        nc.vector.tensor_reduce(zn[:, 1:2], nmc[:], mybir.AxisListType.X, OP.add)
        zn2 = ps.tile([2 * MNODES, 2], F32, tag="sb", name="zn2")
        nc.tensor.matmul(zn2[:], cmb_sb[:], zn[:], start=True, stop=True)
        zr_col = p_sm.tile([2 * MNODES, 1], F32, tag="zrc")
        nc.vector.reciprocal(zr_col[:], zn2[:, 0:1])
        f_col = p_sm.tile([2 * MNODES, 1], F32, tag="fc")
        nc.vector.tensor_mul(f_col[:], zn2[:, 1:2], zr_col[:])
        dctf = p_sm.tile([2 * MNODES, KDEG], F32R, tag="dctf")
        nc.vector.tensor_scalar_mul(dctf[:], dct_sb[:], f_col[:])

        # =========== a path -> tt packed [128, 32] via PE transposes ======
        pa = ps.tile([H, T], F32, tag="sa")
        for kt in range(KC):
            nc.tensor.matmul(pa[:], mp_sb[:, kt * H:(kt + 1) * H], xs_t[kt][:],
                             start=(kt == 0), stop=(kt == KC - 1))
        # tt_sb columns permuted to (lh, b, ll): transposing a contiguous
        # [8, 128] block then yields partitions p = b*64 + ll (batch-major),
        # with matmul/psum outputs at partition base 0 as required.
        tt_sb = p_sm.tile([H, T], F32, tag="tts")
        nc.scalar.activation(
            tt_sb[:],
            pa[:].rearrange("h (b lh ll) -> h lh b ll", b=2, lh=4),
            AF.Identity, bias=a0_sb[:])
        tt_ps = ps.tile([128, 32], F32, tag="sb")
        for lh in range(4):
            nc.tensor.transpose(
                tt_ps[:, lh * 8:(lh + 1) * 8],
                tt_sb[0:8, lh * 128:(lh + 1) * 128],
                ident_sb[0:8, 0:8])

        # =========== Chebyshev fit on PE: ck [2,14] -> cbB [128,14] =======
        ck_ps = ps.tile([2, KDEG], F32, tag="sb", name="ckps")
        nc.tensor.matmul(ck_ps[:], bi_sb[:], dctf[:],
                         start=True, stop=True)
        ck_sb = p_sm.tile([2, KDEG], F32R, tag="cksb")
        nc.scalar.copy(ck_sb[:], ck_ps[:])
        cb_ps = ps.tile([128, KDEG], F32, tag="sb", name="cbps")
        nc.tensor.matmul(cb_ps[:], bi2_sb[:], ck_sb[:],
                         start=True, stop=True)
        cb = p_cl.tile([128, KDEG], F32, tag="cb")
        nc.scalar.copy(cb[:], cb_ps[:])

        def ce(k):  # even coeff c_{2k}
            return cb[:, 2 * k:2 * k + 1]

        def co(k):  # odd coeff c_{2k+1}
            return cb[:, 2 * k + 1:2 * k + 2]

        # =========== xc tiles + early stat matmuls ========================
        xc_t, xc2_t = [], []
        for kt in range(KC):
            xc = p_act.tile([128, T], FP16, tag="xc", bufs=4, name=f"xc{kt}")
            nc.scalar.activation(xc[:], xs_t[kt][:].bitcast(F32), AF.Identity,
                                 bias=cv_c(kt))
            xc_t.append(xc)
        # bulk fp16 weight loads, issued behind xc on the gpsimd queue
        w2_all = p_w2.tile([128, KH * C], FP16, tag="w2")
        for kt in range(KH):
            nc.gpsimd.dma_start(w2_all[:, kt * C:(kt + 1) * C],
                                w2t[kt * 128:(kt + 1) * 128, :])
        wo_all = p_wo.tile([128, KC * C], FP16, tag="wo")
        for kt in range(KC):
            nc.gpsimd.dma_start(wo_all[:, kt * C:(kt + 1) * C],
                                wot[kt * 128:(kt + 1) * 128, :])
        def w2_sl(kt, mt):
            return w2_all[:, kt * C + mt * 128:kt * C + (mt + 1) * 128]

        def wo_sl(kt, mt):
            return wo_all[:, kt * C + mt * 128:kt * C + (mt + 1) * 128]
        warm = p_sm.tile([128, 1], F32, tag="warm")
        nc.scalar.activation(warm[:], eps_col[:], AF.Sqrt)
        # xcu[h, tau] = sum_{c in h} u_v[c] xc[c, tau]
        xcu_ps = ps.tile([H, T], F32, tag="sb")
        for kt in range(KC):
            nc.tensor.matmul(xcu_ps[:], uvmask_sb[:, kt * H:(kt + 1) * H],
                             xc_t[kt][:], start=(kt == 0), stop=(kt == KC - 1))
        # stat0/stat1 partial sums over xc / xc^2 (keep groups open for w terms)
        st0 = ps.tile([1, T], F32, tag="st0")
        st1 = ps.tile([1, T], F32, tag="st1")
        for kt in range(KC):
            nc.tensor.matmul(st0[:], ones_c[:], xc_t[kt][:],
                             start=(kt == 0), stop=False)

        # =========== Clenshaw: dual chains on [128, 32] ===================
        tt = p_cl.tile([128, 32], F32, tag="tt")
        nc.vector.tensor_scalar(tt[:], tt_ps[:], -1.0, 1.0, op0=OP.max, op1=OP.min)
        t2 = p_cl.tile([128, 32], F32, tag="t2")
        nc.vector.tensor_mul(t2[:], tt[:], tt[:])
        u2 = p_cl.tile([128, 32], F32, tag="u2")
        nc.vector.tensor_scalar(u2[:], t2[:], 4.0, -2.0, op0=OP.mult, op1=OP.add)
        uu = p_cl.tile([128, 32], F32, tag="uu")
        nc.vector.tensor_scalar(uu[:], t2[:], 2.0, -1.0, op0=OP.mult, op1=OP.add)

        eb1 = p_cl.tile([128, 32], F32, tag="eb1")
        eb2 = p_cl.tile([128, 32], F32, tag="eb2")
        etm = p_cl.tile([128, 32], F32, tag="etm")
        ob1 = p_cl.tile([128, 32], F32, tag="ob1")
        ob2 = p_cl.tile([128, 32], F32, tag="ob2")
        otm = p_cl.tile([128, 32], F32, tag="otm")
        # init: b_{n-2} = c_{n-2} + 2u*c_{n-1}  (b_{n-1}=c_{n-1}, b_n=0)
        nc.vector.tensor_scalar(eb2[:], u2[:], ce(NE - 1), ce(NE - 2),
                                op0=OP.mult, op1=OP.add)
        nc.vector.tensor_scalar(ob2[:], u2[:], co(NO - 1), co(NO - 2),
                                op0=OP.mult, op1=OP.add)
        # eb2 currently holds b_{n-2}; need previous b_{n-1} too
        nc.vector.tensor_scalar(eb1[:], u2[:], 0.0, ce(NE - 1),
                                op0=OP.mult, op1=OP.add)
        nc.vector.tensor_scalar(ob1[:], u2[:], 0.0, co(NO - 1),
                                op0=OP.mult, op1=OP.add)
        # now (cur1, cur2) = (b_{m+1}, b_{m+2}) with m+1 = n-2 held in (eb2, eb1)
        ecur1, ecur2 = eb2, eb1
        ocur1, ocur2 = ob2, ob1
        for k in range(NE - 3, 0, -1):
            nc.vector.tensor_mul(etm[:], u2[:], ecur1[:])
            nc.vector.tensor_mul(otm[:], u2[:], ocur1[:])
            nc.vector.scalar_tensor_tensor(
                out=ecur2[:], in0=etm[:], scalar=ce(k), in1=ecur2[:],
                op0=OP.add, op1=OP.subtract)
            nc.vector.scalar_tensor_tensor(
                out=ocur2[:], in0=otm[:], scalar=co(k), in1=ocur2[:],
                op0=OP.add, op1=OP.subtract)
            ecur1, ecur2 = ecur2, ecur1
            ocur1, ocur2 = ocur2, ocur1
        # even final: fe = u*b1 - b2 + c0
        fe = p_cl.tile([128, 32], F32, tag="fe")
        nc.vector.tensor_mul(etm[:], uu[:], ecur1[:])
        nc.vector.scalar_tensor_tensor(
            out=fe[:], in0=etm[:], scalar=ce(0), in1=ecur2[:],
            op0=OP.add, op1=OP.subtract)
        # odd final: b0 = c0 + 2u*b1 - b2 ; fo = b0 - b1
        nc.vector.tensor_mul(otm[:], u2[:], ocur1[:])
        nc.vector.scalar_tensor_tensor(
            out=ocur2[:], in0=otm[:], scalar=co(0), in1=ocur2[:],
            op0=OP.add, op1=OP.subtract)
        fo = p_cl.tile([128, 32], F32, tag="fo")
        nc.vector.tensor_sub(fo[:], ocur2[:], ocur1[:])
        # w = fe + t*fo
        w_pack = p_cl.tile([128, 32], F32, tag="wp")
        nc.vector.tensor_mul(otm[:], tt[:], fo[:])
        nc.vector.tensor_add(w_pack[:], fe[:], otm[:])

        for kt in range(KC):
            x2 = p_act.tile([128, T], FP16, tag="xc2", bufs=4, name=f"xc2{kt}")
            nc.vector.tensor_mul(x2[:], xc_t[kt][:], xc_t[kt][:])
            xc2_t.append(x2)
        for kt in range(KC):
            nc.tensor.matmul(st1[:], ones_c[:], xc2_t[kt][:],
                             start=(kt == 0), stop=False)

        # =========== w -> [H, T] via reverse transposes ===================
        # wT columns come out in (lh, b, ll) order; the psum->sbuf copies
        # permute back to (b, lh, ll) = tau order.
        wht_ps = ps.tile([H, T], F32, tag="sa", name="whtps")
        for lh in range(4):
            nc.tensor.transpose(
                wht_ps[0:8, lh * 128:(lh + 1) * 128],
                w_pack[:, lh * 8:(lh + 1) * 8],
                ident_sb[:, :])
        wsrc = wht_ps[:].rearrange("h (lh b ll) -> h b lh ll", b=2, lh=4)
        whth = p_sm.tile([H, T], FP16, tag="whth")
        nc.vector.tensor_copy(whth[:], wsrc)
        w2h = p_sm.tile([H, T], FP16, tag="w2h")
        nc.vector.tensor_mul(w2h[:], whth[:], whth[:])
        wxcu = p_sm.tile([H, T], FP16, tag="wxcu")
        nc.vector.tensor_mul(wxcu[:], whth[:], xcu_ps[:])

        # =========== finish LN1 stats with w-dependent terms ==============
        nc.tensor.matmul(st0[:], spack_sb[:, 0:1], whth[:], start=False, stop=True)
        nc.tensor.matmul(st1[:], spack_sb[:, 1:2], w2h[:], start=False, stop=False)
        nc.tensor.matmul(st1[:], spack_sb[:, 2:3], wxcu[:], start=False, stop=True)

        def ln_front(st0_ps, st1_ps, ph):
            """stats psums [1,T] (mu, E[y^2]) -> (smu [1,T] f32r, rstdp
            [128,4] fp16). rstd is computed in a transposed [128,4] packing so
            the reciprocal runs at 4 elems/lane instead of 512."""
            smu = p_sm.tile([1, T], F32, tag=f"smu{ph}")
            nc.scalar.copy(smu[:], st0_ps[:])
            mu16 = p_sm.tile([1, T], FP16, tag=f"mu16{ph}")
            nc.scalar.copy(mu16[:], st0_ps[:])
            ss1 = p_sm.tile([1, T], F32, tag=f"ss1{ph}")
            nc.vector.tensor_copy(ss1[:], st1_ps[:])
            lnpmu = ps.tile([128, 4], F32, tag="sa", name=f"lnpmu{ph}")
            for r in range(4):
                nc.tensor.transpose(lnpmu[:, r:r + 1],
                                    smu[0:1, r * 128:(r + 1) * 128],
                                    ident_sb[0:1, 0:1])
            lnps1 = ps.tile([128, 4], F32, tag="sb", name=f"lnps1{ph}")
            for r in range(4):
                nc.tensor.transpose(lnps1[:, r:r + 1],
                                    ss1[0:1, r * 128:(r + 1) * 128],
                                    ident_sb[0:1, 0:1])
            mup = p_sm.tile([128, 4], F32, tag=f"mup{ph}")
            nc.scalar.copy(mup[:], lnpmu[:])
            musq = p_sm.tile([128, 4], F32, tag=f"msq{ph}")
            nc.vector.tensor_mul(musq[:], mup[:], mup[:])
            var = p_sm.tile([128, 4], F32, tag=f"var{ph}")
            nc.vector.tensor_sub(var[:], lnps1[:], musq[:])
            stdp = p_sm.tile([128, 4], FP16, tag=f"stdp{ph}")
            nc.scalar.activation(stdp[:], var[:], AF.Sqrt, bias=eps_col[:])
            rstdp = p_sm.tile([128, 4], FP16, tag=f"rsp{ph}")
            with nc.allow_low_precision(reason="fp16 rstd is plenty for LN"):
                nc.vector.reciprocal(rstdp[:], stdp[:])
            return mu16, rstdp

        def ln_back(rstdp, ph):
            rsT = ps.tile([4, 128], FP16, tag="sb", name=f"rsT{ph}")
            nc.tensor.transpose(rsT[:], rstdp[:], id16_sb[:, :])
            rsrow = p_sm.tile([4, 128], FP16, tag=f"rsr{ph}")
            nc.scalar.copy(rsrow[:], rsT[:])
            # chunk r of R selects rstd row r via a one-hot lhsT
            rps = ps.tile([128, T], F32, tag="bcR", name=f"rps{ph}")
            for r in range(4):
                nc.tensor.matmul(rps[:, r * 128:(r + 1) * 128],
                                 ohot_sb[:, r * 128:(r + 1) * 128],
                                 rsrow[:], start=True, stop=True)
            return rps

        with tc.high_priority():
            mu16a, rstdp1 = ln_front(st0, st1, "a")
            rps1 = ln_back(rstdp1, "a")

        # =========== ym = y - mu accumulated on the PE; xhat = ym * rstd ==
        xh_t, x_t = [], []
        for kt in range(KC):
            ym = ps.tile([128, T], F32, tag="mm", bufs=3, name=f"ym{kt}")
            nc.tensor.matmul(ym[:], uvrep_sb[:, kt * 128:(kt + 1) * 128],
                             whth[:], start=True, stop=False)
            nc.tensor.matmul(ym[:], id16_sb[:, :], xc_t[kt][:],
                             start=False, stop=False)
            nc.tensor.matmul(ym[:], nones_pe[:], mu16a[:],
                             start=False, stop=True)
            ysb = p_act.tile([128, T], FP16, tag="ysb", bufs=4, name=f"ysb{kt}")
            nc.scalar.copy(ysb[:], ym[:])
            xh = p_act.tile([128, T], FP16, tag="xh", bufs=4, name=f"xh{kt}")
            nc.vector.tensor_mul(xh[:], ysb[:], rps1[:])
            xh_t.append(xh)
        # x = g1*xhat + be1 (residual input), off critical path
        for kt in range(KC):
            xk = p_act.tile([128, T], FP16, tag="x", bufs=4, name=f"x{kt}")
            nc.scalar.activation(xk[:], xh_t[kt][:], AF.Identity,
                                 scale=g1_c(kt), bias=be1_c(kt))
            x_t.append(xk)

        # =========== FFN1: h = relu(W1g @ xhat + b1f) =====================
        h_t = []
        for mt in range(KH):
            sl = slice(mt * 128, (mt + 1) * 128)
            pf = ps.tile([128, T], F32, tag="mm", bufs=3, name=f"pf1{mt}")
            for kt in range(KC):
                nc.tensor.matmul(pf[:], w1_t[kt][:, sl], xh_t[kt][:],
                                 start=(kt == 0), stop=(kt == KC - 1))
            hm = p_h.tile([128, T], FP16, tag="h", name=f"h{mt}")
            nc.scalar.activation(hm[:], pf[:], AF.Relu,
                                 bias=b1f_sb[:, mt:mt + 1])
            h_t.append(hm)

        # =========== FFN2 + residual -> y2 ================================
        y2_t, sq2_t = [], []
        for mt in range(KC):
            sl = slice(mt * 128, (mt + 1) * 128)
            pf = ps.tile([128, T], F32, tag="mm", bufs=3, name=f"pf2{mt}")
            for kt in range(KH):
                nc.tensor.matmul(pf[:], w2_sl(kt, mt), h_t[kt][:],
                                 start=(kt == 0), stop=(kt == KH - 1))
            y2 = p_act.tile([128, T], FP16, tag="y2", bufs=4, name=f"y2{mt}")
            nc.vector.scalar_tensor_tensor(
                out=y2[:], in0=x_t[mt][:], scalar=b2_c(mt), in1=pf[:],
                op0=OP.add, op1=OP.add)
            y2_t.append(y2)
            sq = p_act.tile([128, T], FP16, tag="sq2", bufs=4, name=f"sq2{mt}")
            nc.vector.tensor_mul(sq[:], y2[:], y2[:])
            sq2_t.append(sq)

        # =========== LN2 stats + zhat =====================================
        st0b = ps.tile([1, T], F32, tag="st0", name="st0b")
        st1b = ps.tile([1, T], F32, tag="st1", name="st1b")
        for kt in range(KC):
            nc.tensor.matmul(st0b[:], ones_c[:], y2_t[kt][:],
                             start=(kt == 0), stop=(kt == KC - 1))
        for kt in range(KC):
            nc.tensor.matmul(st1b[:], ones_c[:], sq2_t[kt][:],
                             start=(kt == 0), stop=(kt == KC - 1))
        with tc.high_priority():
            mu16b, rstdp2 = ln_front(st0b, st1b, "b")
            rps2 = ln_back(rstdp2, "b")
        zh_t = []
        for kt in range(KC):
            ym2 = ps.tile([128, T], F32, tag="mm", bufs=3, name=f"ym2{kt}")
            nc.tensor.matmul(ym2[:], id16_sb[:, :], y2_t[kt][:],
                             start=True, stop=False)
            nc.tensor.matmul(ym2[:], nones_pe[:], mu16b[:],
                             start=False, stop=True)
            y2sb = p_act.tile([128, T], FP16, tag="ysb", bufs=4, name=f"y2sb{kt}")
            nc.scalar.copy(y2sb[:], ym2[:])
            zh = p_act.tile([128, T], FP16, tag="zh", bufs=4, name=f"zh{kt}")
            nc.vector.tensor_mul(zh[:], y2sb[:], rps2[:])
            zh_t.append(zh)

        # =========== out = Wo_g @ zhat + bof ==============================
        oq = [nc.sync, nc.scalar, nc.sync, nc.scalar]
        for mt in range(KC):
            sl = slice(mt * 128, (mt + 1) * 128)
            pf = ps.tile([128, T], F32, tag="mm", bufs=3, name=f"pfo{mt}")
            for kt in range(KC):
                nc.tensor.matmul(pf[:], wo_sl(kt, mt), zh_t[kt][:],
                                 start=(kt == 0), stop=(kt == KC - 1))
            om = p_act.tile([128, T], F32, tag="om", bufs=2, name=f"om{mt}")
            for b in range(B):
                sl2 = slice(b * LC, (b + 1) * LC)
                if (2 * mt + b) % 2 == 0:
                    nc.scalar.activation(om[:, sl2], pf[:, sl2], AF.Identity,
                                         bias=bof_c(mt))
                else:
                    nc.vector.tensor_scalar_add(om[:, sl2], pf[:, sl2],
                                                bof_c(mt))
                dq = nc.sync if b == 0 else nc.scalar
                dq.dma_start(out_sl[b, mt * 128:(mt + 1) * 128, :], om[:, sl2])

    nc.compile()
    return nc


def _host_prep(inputs):
    f32 = lambda x: np.ascontiguousarray(np.asarray(x), dtype=np.float32)
    f16 = lambda x: np.ascontiguousarray(np.asarray(x), dtype=np.float16)
    Wq, Wk, Wv = f32(inputs["Wq"]), f32(inputs["Wk"]), f32(inputs["Wv"])
    Wg, bg = f32(inputs["Wg"]), f32(inputs["bg"])
    bq, bv = f32(inputs["bq"]), f32(inputs["bv"])
    W1, b1, W2, b2 = (f32(inputs["W1"]), f32(inputs["b1"]),
                      f32(inputs["W2"]), f32(inputs["b2"]))
    Wo, bo = f32(inputs["Wo"]), f32(inputs["bo"])
    g1, be1 = f32(inputs["g1"]), f32(inputs["beta1"])
    g2, be2 = f32(inputs["g2"]), f32(inputs["beta2"])

    u_k = Wk @ Wg[:, 0]
    u_v = Wv @ Wg[:, 0]
    c_v = Wv @ bg + bv
    hidx = np.arange(C) // D                       # head of channel c
    U = np.zeros((C, H), np.float32)
    U[np.arange(C), hidx] = u_k
    Mp = (Wq.T @ U) * (SCALE / SCAL)               # [C, H]
    a0 = (bq @ U) * (SCALE / SCAL)                 # [H]

    # packed M': [128, (kt, h)]
    mp = np.zeros((128, KC * H), np.float32)
    for kt in range(KC):
        mp[:, kt * H:(kt + 1) * H] = Mp[kt * 128:(kt + 1) * 128, :]
    uvrep = np.zeros((H, C), np.float32)
    uvrep[hidx, np.arange(C)] = u_v
    uvmask = np.zeros((128, KC * H), np.float32)
    UV = np.zeros((C, H), np.float32)
    UV[np.arange(C), hidx] = u_v
    for kt in range(KC):
        uvmask[:, kt * H:(kt + 1) * H] = UV[kt * 128:(kt + 1) * 128, :]
    s_uv = UV.sum(axis=0) / C                      # [H]
    s_uv2 = (UV * UV).sum(axis=0) / C
    spack = np.stack([s_uv, s_uv2, np.full(H, 2.0 / C)], axis=1)

    W1f = W1 * g1[None, :]
    b1f = b1 + W1 @ be1
    Wof = Wo * g2[None, :]
    bof = bo + Wo @ be2

    vecp = np.zeros((128, KC * 5), np.float32)
    for kt in range(KC):
        sl = slice(kt * 128, (kt + 1) * 128)
        for i, v in enumerate([c_v, b2, bof, g1, be1]):
            vecp[:, kt * 5 + i] = v[sl]
    b1fp = np.zeros((128, KH), np.float32)
    for mt in range(KH):
        b1fp[:, mt] = b1f[mt * 128:(mt + 1) * 128]

    return {
        "expv": f32(inputs["exp"]),
        "w1t": f16(W1f.T),
        "w2t": f16(W2.T),
        "wot": f16(Wof.T),
        "mp": mp,
        "a0p": np.ascontiguousarray(a0[:, None]),
        "uvrep": f16(uvrep),
        "uvmask": f16(uvmask),
        "spack": f16(spack),
        "vecp": vecp,
        "b1fp": b1fp,
    }


def kernel(**inputs):
    global _CACHE, LAST_RESULTS
    if _CACHE is None:
        _CACHE = _build()
    nc = _CACHE

    base = _host_prep(inputs)
    seq = np.ascontiguousarray(np.asarray(inputs["seq"]), dtype=np.float32)
    in_maps = []
    for c in range(NCORES):
        m = dict(base)
        m["seq_sl"] = np.ascontiguousarray(seq[:, :, c * LC:(c + 1) * LC])
        in_maps.append(m)

    res = run_bass_kernel_spmd(nc, in_maps, list(range(NCORES)), trace=TRACE,
                               **TRACE_KW)
    LAST_RESULTS = res
    out = np.empty((B, C, L), np.float32)
    for c in range(NCORES):
        out[:, :, c * LC:(c + 1) * LC] = res.results[c]["out_sl"]
    return out
